# revision 39
# baseline (speedup 1.0000x reference)
"""Bass/Tile TRN2 kernel for nn_LocalNodeAttentionHead.

Reference computation (per sample b):
    xi = x[:, :, t0]  (center frame)          (C, HW)
    xw = x reshaped                           (C, L)    L = T*H*W
    q  = Wq @ xi + bq                         (CI, HW)
    k  = Wk @ xw + bk                         (CI, L)
    v  = Wv @ xw + bv                         (L, CI)
    S  = q^T k  -> softmax over L             (HW, L)
    y  = softmax(S) @ v                       (CI, HW)
    out = Wo @ y + bo + xi                    (C, HW)

Algebraic restructure (host folds the weight products, fp32):
    S   = q^T Wk x = qt^T x    with qt = W1 xi + b1, W1 = Wk^T Wq, b1 = Wk^T bq
          (bk drops: constant along l under softmax)
    out = Wo (P V) + bo + xi = W2 (X P^T) + xib
          with W2 = Wo Wv, xib = xi + bo + Wo bv
          (rows of P sum to 1, so bv leaves the attention sum as a constant)
The k/v projections disappear entirely; per-sample device work is four
small GEMM groups: qt (16 mm), scores (28 mm), Z = X P^T (56 mm + 28 PE
transposes), out = W2 Z (16 mm).

Softmax runs chunked: each 441-wide score chunk takes its own max and
exponentiates straight out of PSUM; the cross-chunk correction
gamma_c = exp(cmax_c - M) and the 1/rowsum normalization combine into one
per-chunk scale applied to P in place on DVE before the PE transposes
(the PE transpose datapath only accepts permutation ifmaps).

Distribution: pure data-parallel, 4 samples per core on 8 cores.
All matmul operands fp16 (full PE rate at any free size), PSUM fp32,
softmax statistics fp32, residual fp16 (quantization ~2e-4 of scale).
x is shipped in both (C,L) and (L,C) layouts so the Z contraction needs
no on-chip transpose of x; the kv axis is permuted center-frame-first so
every DMA piece is a contiguous HBM block, pieces alternate between the
SP and ACT queues, and the output projection for channels 0-255 opens
mid-way through the second Z pass to hide its PE time.
"""

import sys

sys.path.insert(0, "/opt/trn_rl_repo")

import numpy as np

import concourse.bass as bass
import concourse.tile as tile
from concourse import bacc, mybir

F32 = mybir.dt.float32
F16 = mybir.dt.float16
AF = mybir.ActivationFunctionType
AX = mybir.AxisListType.X
ALU = mybir.AluOpType

B, C, T, H, W = 32, 512, 9, 14, 14
CI = 512
HWm = H * W  # 196
L = T * HWm  # 1764
CENT = (T // 2) * HWm  # 784, center-frame offset in L
NCORES = 8
BC = B // NCORES  # 4 samples per core

NCH = C // 128  # 4 chunks of the channel dims
MC = 98  # query-row chunk (2 chunks of HW=196)
NMC = HWm // MC
SC = 441  # score l-chunk (PSUM fp32: 1764B < one bank)
NSC = L // SC  # 4
LV = 126  # l-chunk for P^T / Z (14 chunks; transpose out partitions)
NLV = L // LV
# The kv positions l are PERMUTED center-frame-first on the host (the
# attention math is order-invariant over l as long as x_cl, x_lc and P
# share the ordering): the qt projection reads one contiguous slice of
# DMA piece 0, and every x_cl piece is a fully contiguous HBM block.
CHUNK_ORDER = list(range(NSC))


def build_program():
    nc = bacc.Bacc("TRN2", target_bir_lowering=False, debug=False)

    xcl = nc.dram_tensor(
        "xcl", [BC, NSC, 128, NCH, SC], F16, kind="ExternalInput"
    ).ap()
    xlc = nc.dram_tensor("xlc", [BC, LV, NLV, C], F16, kind="ExternalInput").ap()
    w1T = nc.dram_tensor("w1T", [128, NCH, C], F16, kind="ExternalInput").ap()
    w2T = nc.dram_tensor("w2T", [128, NCH, C], F16, kind="ExternalInput").ap()
    b1 = nc.dram_tensor("b1", [128, NCH], F32, kind="ExternalInput").ap()
    xib = nc.dram_tensor("xib", [128, NCH, BC, HWm], F16, kind="ExternalInput").ap()
    ident = nc.dram_tensor("ident", [MC, MC], F16, kind="ExternalInput").ap()
    out = nc.dram_tensor("out", [BC, C, HWm], F16, kind="ExternalOutput").ap()

    with tile.TileContext(nc) as tc:
        with (
            tc.tile_pool(name="const", bufs=1) as const,
            tc.tile_pool(name="sb", bufs=1) as sb,
            tc.tile_pool(name="ps", bufs=1, space="PSUM") as ps,
        ):
            # ---- constants: qt-path weights lead the SP queue; the
            # phase_b-only constants head the pool queue ahead of x_lc
            w1_sb = const.tile([128, NCH, C], F16)
            nc.sync.dma_start(w1_sb[:], w1T[:])
            b1_sb = const.tile([128, NCH], F32)
            nc.sync.dma_start(b1_sb[:], b1[:])
            id_sb = const.tile([MC, MC], F16)
            nc.gpsimd.dma_start(id_sb[:], ident[:])
            w2_sb = const.tile([128, NCH, C], F16)
            nc.gpsimd.dma_start(w2_sb[:], w2T[:])
            xib_sb = const.tile([128, NCH, BC, HWm], F16)
            nc.gpsimd.dma_start(xib_sb[:], xib[:])

            state = {}

            def emit_dma(s):
                # interleave each sample's pieces across both the SP and
                # ACT DMA queues so every sample's x arrives in roughly half
                # the single-queue serial time (helps the startup samples)
                xcl_t = sb.tile(
                    [128, NSC, NCH, SC], F16, tag="xcl", bufs=3, name="xcl_t"
                )
                for ci, c in enumerate(CHUNK_ORDER):
                    xq = nc.sync if ci % 2 == 0 else nc.scalar
                    xq.dma_start(xcl_t[:, c], xcl[s][c])
                xlc_t = sb.tile([LV, NLV, C], F16, tag="xlc", bufs=3, name="xlc_t")
                for h in range(2):
                    nc.gpsimd.dma_start(
                        xlc_t[:, h * 7 : (h + 1) * 7, :],
                        xlc[s][:, h * 7 : (h + 1) * 7, :],
                    )
                state[s] = {"xcl": xcl_t, "xlc": xlc_t}

            def phase_a(s):
                st = state[s]
                xcl_t = st["xcl"]
                # qt = W1 @ xi + b1, output (C-chunk partition, HW free)
                qt = sb.tile([128, NCH, HWm], F16, tag="qt", bufs=2, name="qt")
                for pair in range(2):
                    qp = ps.tile([128, 2, HWm], F32, tag="pA", bufs=3, name="qp")
                    for i in range(2):
                        mk = 2 * pair + i
                        for cj in range(NCH):
                            nc.tensor.matmul(
                                qp[:, i, :],
                                w1_sb[:, cj, mk * 128 : (mk + 1) * 128],
                                xcl_t[:, 0, cj, 0:HWm],
                                start=(cj == 0),
                                stop=(cj == NCH - 1),
                            )
                    for i in range(2):
                        mk = 2 * pair + i
                        nc.scalar.activation(
                            qt[:, mk, :],
                            qp[:, i, :],
                            AF.Identity,
                            bias=b1_sb[:, mk : mk + 1],
                        )

                # scores qt^T x, chunked; per-chunk max + exp from PSUM
                p_t = sb.tile([MC, NMC, L], F16, tag="p", bufs=2, name="p_t")
                ncm = sb.tile([MC, NMC, NSC], F32, tag="ncm", bufs=2, name="ncm")
                rsums = sb.tile(
                    [MC, NMC, NSC], F32, tag="rsums", bufs=2, name="rsums"
                )
                for c in CHUNK_ORDER:
                    for mc in range(NMC):
                        sp = ps.tile([MC, SC], F32, tag="pA", bufs=3, name="sp")
                        for cj in range(NCH):
                            nc.tensor.matmul(
                                sp[:],
                                qt[:, cj, mc * MC : (mc + 1) * MC],
                                xcl_t[:, c, cj, :],
                                start=(cj == 0),
                                stop=(cj == NCH - 1),
                            )
                        nc.vector.reduce_max(
                            ncm[:, mc, c : c + 1], sp[:], axis=AX, negate=True
                        )
                        nc.scalar.activation(
                            p_t[:, mc, c * SC : (c + 1) * SC],
                            sp[:],
                            AF.Exp,
                            bias=ncm[:, mc, c : c + 1],
                            accum_out=rsums[:, mc, c : c + 1],
                        )

                st["p"] = p_t
                st["ncm"] = ncm
                st["rsums"] = rsums

            def emit_stats(s):
                # combine chunk stats: scl_c = exp(cmax_c - M) / rowsum.
                # Emitted AFTER phase_b(s-1) so these DVE/ACT ops (which wait
                # on exp(s)) never block the previous sample's scale/copies
                # in the in-order engine streams.
                st = state[s]
                ncm, rsums = st["ncm"], st["rsums"]
                cm2 = sb.tile([MC, NMC, NSC], F32, tag="cm2", bufs=2, name="cm2")
                gam = sb.tile([MC, NMC, NSC], F32, tag="gam", bufs=2, name="gam")
                prod = sb.tile([MC, NMC, NSC], F32, tag="prod", bufs=2, name="prod")
                stat = sb.tile([MC, NMC, 4], F32, tag="stat", bufs=2, name="stat")
                for mc in range(NMC):
                    nc.vector.tensor_scalar_mul(cm2[:, mc, :], ncm[:, mc, :], -1.0)
                    nc.vector.reduce_max(
                        stat[:, mc, 0:1], cm2[:, mc, :], axis=AX, negate=True
                    )
                    nc.scalar.activation(
                        gam[:, mc, :],
                        ncm[:, mc, :],
                        AF.Exp,
                        bias=stat[:, mc, 0:1],
                        scale=-1.0,
                    )
                    # tensor_tensor_reduce hangs real HW; use mul + reduce
                    nc.vector.tensor_mul(
                        prod[:, mc, :], gam[:, mc, :], rsums[:, mc, :]
                    )
                    nc.vector.reduce_sum(
                        stat[:, mc, 1:2], prod[:, mc, :], axis=AX
                    )
                    nc.vector.reciprocal(stat[:, mc, 2:3], stat[:, mc, 1:2])
                    nc.vector.tensor_scalar_mul(
                        gam[:, mc, :], gam[:, mc, :], stat[:, mc, 2:3]
                    )
                # normalize P in place per chunk: P *= gamma_c / rowsum
                # (per-partition scalar; PE transpose mode only passes data
                # through, so scaling must happen before the transposes).
                # Emitted HERE — after the previous sample's z/residual
                # copies and before the next sample's reduce_max ops — so
                # the in-order DVE stream never makes phase_b wait on it.
                p_t = st["p"]
                last = s == BC - 1
                for c in range(NSC):
                    for mc in range(NMC):
                        # on the last sample there is no next phase_a to hide
                        # this chain behind: split it across DVE and ACT
                        # (both idle at the tail) to halve its serial time
                        if last and (c + mc) % 2 == 1:
                            nc.scalar.mul(
                                p_t[:, mc, c * SC : (c + 1) * SC],
                                p_t[:, mc, c * SC : (c + 1) * SC],
                                gam[:, mc, c : c + 1],
                            )
                        else:
                            nc.vector.tensor_scalar_mul(
                                p_t[:, mc, c * SC : (c + 1) * SC],
                                p_t[:, mc, c * SC : (c + 1) * SC],
                                gam[:, mc, c : c + 1],
                            )

            def phase_b(s):
                st = state[s]
                p_t, xlc_t = st["p"], st["xlc"]
                # P^T via PE transpose (identity ifmap), buffered whole in
                # SBUF; Z = X P^T accumulated in two passes of 2 C-chunks so
                # each PSUM bank holds exactly one open accumulation group
                ptsb = sb.tile(
                    [LV, NLV, HWm], F16, tag="ptsb", bufs=2, name="ptsb"
                )
                z_sb = sb.tile([128, NCH, HWm], F16, tag="z", bufs=2, name="z_sb")
                ops = []
                for half in range(2):
                    zta = ps.tile([128, HWm], F32, tag="zt", bufs=2, name="zta")
                    ztb = ps.tile([128, HWm], F32, tag="zt", bufs=2, name="ztb")
                    for lp in range(NLV // 2):
                        if half == 0:
                            ptp = ps.tile(
                                [LV, 2, HWm], F16, tag="pB", bufs=3, name="ptp"
                            )
                            for j in range(2):
                                lc = 2 * lp + j
                                for mc in range(NMC):
                                    nc.tensor.transpose(
                                        ptp[:, j, mc * MC : (mc + 1) * MC],
                                        p_t[:, mc, lc * LV : (lc + 1) * LV],
                                        id_sb[:],
                                    )
                            nc.scalar.copy(
                                ptsb[:, 2 * lp : 2 * lp + 2, :], ptp[:]
                            )
                        elif lp == 2:
                            # open partial out-proj groups for cc 0,1 on the
                            # z chunks already drained from half 0: the PE
                            # work hides inside Z half 1, shortening the tail
                            for cc in range(2):
                                op = ps.tile(
                                    [128, HWm], F32, tag="pB", bufs=3, name="op"
                                )
                                for zj in range(2):
                                    nc.tensor.matmul(
                                        op[:],
                                        w2_sb[:, zj, cc * 128 : (cc + 1) * 128],
                                        z_sb[:, zj, :],
                                        start=(zj == 0),
                                        stop=False,
                                    )
                                ops.append(op)
                        for j in range(2):
                            lc = 2 * lp + j
                            for i, zt in ((0, zta), (1, ztb)):
                                cj = 2 * half + i
                                nc.tensor.matmul(
                                    zt[:],
                                    xlc_t[:, lc, cj * 128 : (cj + 1) * 128],
                                    ptsb[:, lc, :],
                                    start=(lc == 0),
                                    stop=(lc == NLV - 1),
                                )
                    for i, zt in ((0, zta), (1, ztb)):
                        cj = 2 * half + i
                        nc.vector.tensor_copy(z_sb[:, cj, :], zt[:])
                # close cc 0,1 with the second z half; cc 2,3 run whole
                for cc in range(2):
                    for zj in (2, 3):
                        nc.tensor.matmul(
                            ops[cc],
                            w2_sb[:, zj, cc * 128 : (cc + 1) * 128],
                            z_sb[:, zj, :],
                            start=False,
                            stop=(zj == NCH - 1),
                        )
                for cc in (2, 3):
                    op = ps.tile([128, HWm], F32, tag="pB", bufs=3, name="op")
                    for zj in range(NCH):
                        nc.tensor.matmul(
                            op[:],
                            w2_sb[:, zj, cc * 128 : (cc + 1) * 128],
                            z_sb[:, zj, :],
                            start=(zj == 0),
                            stop=(zj == NCH - 1),
                        )
                    ops.append(op)
                for pair in range(2):
                    osb = sb.tile([128, 2, HWm], F16, tag="osb", bufs=2, name="osb")
                    for i in range(2):
                        cc = 2 * pair + i
                        nc.vector.tensor_add(
                            osb[:, i, :], ops[cc][:], xib_sb[:, cc, s, :]
                        )
                    nc.sync.dma_start(
                        out[s].rearrange("(j p) m -> p j m", p=128)[
                            :, 2 * pair : 2 * pair + 2, :
                        ],
                        osb[:],
                    )
                del state[s]

            # prefetch one sample ahead: dispatches sit early in the engine
            # streams without flooding HBM with all samples at once
            emit_dma(0)
            emit_dma(1)
            for s in range(BC):
                if s + 2 < BC:
                    emit_dma(s + 2)
                phase_a(s)
                if s > 0:
                    phase_b(s - 1)
                emit_stats(s)
            phase_b(BC - 1)

    nc.compile()
    return nc


_NC = None


def _get_program():
    global _NC
    if _NC is None:
        _NC = build_program()
    return _NC


def make_in_maps(inputs):
    x_window = np.ascontiguousarray(np.asarray(inputs["x_window"], dtype=np.float32))
    Wq = np.asarray(inputs["Wq"], dtype=np.float32)
    bq_ = np.asarray(inputs["bq"], dtype=np.float32)
    Wk = np.asarray(inputs["Wk"], dtype=np.float32)
    Wv = np.asarray(inputs["Wv"], dtype=np.float32)
    bv_ = np.asarray(inputs["bv"], dtype=np.float32)
    Wo = np.asarray(inputs["Wo"], dtype=np.float32)
    bo_ = np.asarray(inputs["bo"], dtype=np.float32)

    xw = x_window.reshape(B, C, L)
    xib_full = (
        xw[:, :, CENT : CENT + HWm] + bo_[None, :, None] + (Wo @ bv_)[None, :, None]
    )

    def tile_w(wt):  # (in, out) -> [128, NCH, out] partition-major, fp16
        return np.ascontiguousarray(
            wt.reshape(NCH, 128, -1).transpose(1, 0, 2).astype(np.float16)
        )

    shared = {
        "w1T": tile_w(Wq.T @ Wk),
        "w2T": tile_w(Wv.T @ Wo.T),
        "b1": np.ascontiguousarray((Wk.T @ bq_).reshape(NCH, 128).T),
        "ident": np.eye(MC, dtype=np.float16),
    }
    in_maps = []
    for i in range(NCORES):
        m = dict(shared)
        xc = xw[i * BC : (i + 1) * BC]  # (BC, C, L)
        xp = np.concatenate(
            [
                xc[:, :, CENT : CENT + HWm],
                xc[:, :, :CENT],
                xc[:, :, CENT + HWm :],
            ],
            axis=2,
        ).astype(np.float16)  # l permuted center-first
        m["xcl"] = np.ascontiguousarray(
            xp.reshape(BC, NCH, 128, NSC, SC).transpose(0, 3, 2, 1, 4)
        )
        m["xlc"] = np.ascontiguousarray(
            xp.transpose(0, 2, 1).reshape(BC, NLV, LV, C).transpose(0, 2, 1, 3)
        )
        m["xib"] = np.ascontiguousarray(
            xib_full[i * BC : (i + 1) * BC]
            .reshape(BC, NCH, 128, HWm)
            .transpose(2, 1, 0, 3)
            .astype(np.float16)
        )
        in_maps.append(m)
    return in_maps


def run(inputs, trace=False, tmpdir=None):
    from concourse.bass_utils import run_bass_kernel_spmd

    nc = _get_program()
    in_maps = make_in_maps(inputs)
    res = run_bass_kernel_spmd(
        nc, in_maps, core_ids=list(range(NCORES)), trace=trace, tmpdir=tmpdir
    )
    outs = np.stack(
        [res.results[i]["out"].astype(np.float32) for i in range(NCORES)]
    )  # (8,4,C,HW)
    full = outs.reshape(B, C, HWm).reshape(B, C, 1, H, W)
    return full, res


def kernel(**inputs):
    full, _ = run(inputs)
    return full


# revision 41
# speedup vs baseline: 1.0400x; 1.0400x over previous
"""Bass/Tile TRN2 kernel for nn_LocalNodeAttentionHead.

Reference computation (per sample b):
    xi = x[:, :, t0]  (center frame)          (C, HW)
    xw = x reshaped                           (C, L)    L = T*H*W
    q  = Wq @ xi + bq                         (CI, HW)
    k  = Wk @ xw + bk                         (CI, L)
    v  = Wv @ xw + bv                         (L, CI)
    S  = q^T k  -> softmax over L             (HW, L)
    y  = softmax(S) @ v                       (CI, HW)
    out = Wo @ y + bo + xi                    (C, HW)

Algebraic restructure (host folds the weight products, fp32):
    S   = q^T Wk x = qt^T x    with qt = W1 xi + b1, W1 = Wk^T Wq, b1 = Wk^T bq
          (bk drops: constant along l under softmax)
    out = Wo (P V) + bo + xi = W2 (X P^T) + xib
          with W2 = Wo Wv, xib = xi + bo + Wo bv
          (rows of P sum to 1, so bv leaves the attention sum as a constant)
The k/v projections disappear entirely; per-sample device work is four
small GEMM groups: qt (16 mm), scores (28 mm), Z = X P^T (56 mm + 28 PE
transposes), out = W2 Z (16 mm).

Softmax runs chunked: each 441-wide score chunk takes its own max and
exponentiates straight out of PSUM; the cross-chunk correction
gamma_c = exp(cmax_c - M) and the 1/rowsum normalization combine into one
per-chunk scale applied to P in place on DVE before the PE transposes
(the PE transpose datapath only accepts permutation ifmaps).

Distribution: pure data-parallel, 4 samples per core on 8 cores.
All matmul operands fp16 (full PE rate at any free size), PSUM fp32,
softmax statistics fp32, residual fp16 (quantization ~2e-4 of scale).
x is shipped in both (C,L) and (L,C) layouts so the Z contraction needs
no on-chip transpose of x.
"""

import sys

sys.path.insert(0, "/opt/trn_rl_repo")

import numpy as np

import concourse.bass as bass
import concourse.tile as tile
from concourse import bacc, mybir

F32 = mybir.dt.float32
F16 = mybir.dt.float16
AF = mybir.ActivationFunctionType
AX = mybir.AxisListType.X
ALU = mybir.AluOpType

B, C, T, H, W = 32, 512, 9, 14, 14
CI = 512
HWm = H * W  # 196
L = T * HWm  # 1764
CENT = (T // 2) * HWm  # 784, center-frame offset in L
NCORES = 8
BC = B // NCORES  # 4 samples per core

NCH = C // 128  # 4 chunks of the channel dims
MC = 98  # query-row chunk (2 chunks of HW=196)
NMC = HWm // MC
SC = 441  # score l-chunk (PSUM fp32: 1764B < one bank)
NSC = L // SC  # 4
LV = 126  # l-chunk for P^T / Z (14 chunks; transpose out partitions)
NLV = L // LV
# The kv positions l are PERMUTED center-frame-first on the host (the
# attention math is order-invariant over l as long as x_cl, x_lc and P
# share the ordering): the qt projection reads one contiguous slice of
# DMA piece 0, and every x_cl piece is a fully contiguous HBM block.
CHUNK_ORDER = list(range(NSC))


def build_program():
    nc = bacc.Bacc("TRN2", target_bir_lowering=False, debug=False)

    xcl = nc.dram_tensor(
        "xcl", [BC, NSC, 128, NCH, SC], F16, kind="ExternalInput"
    ).ap()
    xlc = nc.dram_tensor("xlc", [BC, LV, NLV, C], F16, kind="ExternalInput").ap()
    w1T = nc.dram_tensor("w1T", [128, NCH, C], F16, kind="ExternalInput").ap()
    w2T = nc.dram_tensor("w2T", [128, NCH, C], F16, kind="ExternalInput").ap()
    b1 = nc.dram_tensor("b1", [128, NCH], F32, kind="ExternalInput").ap()
    xib = nc.dram_tensor("xib", [128, NCH, BC, HWm], F16, kind="ExternalInput").ap()
    ident = nc.dram_tensor("ident", [MC, MC], F16, kind="ExternalInput").ap()
    out = nc.dram_tensor("out", [BC, C, HWm], F16, kind="ExternalOutput").ap()

    with tile.TileContext(nc) as tc:
        with (
            tc.tile_pool(name="const", bufs=1) as const,
            tc.tile_pool(name="sb", bufs=1) as sb,
            tc.tile_pool(name="ps", bufs=1, space="PSUM") as ps,
        ):
            # ---- constants: qt-path weights lead the SP queue; the
            # phase_b-only constants head the pool queue ahead of x_lc
            w1_sb = const.tile([128, NCH, C], F16)
            nc.sync.dma_start(w1_sb[:], w1T[:])
            b1_sb = const.tile([128, NCH], F32)
            nc.sync.dma_start(b1_sb[:], b1[:])
            id_sb = const.tile([MC, MC], F16)
            nc.gpsimd.dma_start(id_sb[:], ident[:])
            w2_sb = const.tile([128, NCH, C], F16)
            nc.gpsimd.dma_start(w2_sb[:], w2T[:])
            xib_sb = const.tile([128, NCH, BC, HWm], F16)
            nc.gpsimd.dma_start(xib_sb[:], xib[:])

            state = {}

            def emit_dma(s):
                # interleave each sample's pieces across both the SP and
                # ACT DMA queues so every sample's x arrives in roughly half
                # the single-queue serial time (helps the startup samples)
                xcl_t = sb.tile(
                    [128, NSC, NCH, SC], F16, tag="xcl", bufs=3, name="xcl_t"
                )
                for ci, c in enumerate(CHUNK_ORDER):
                    xq = nc.sync if ci % 2 == 0 else nc.scalar
                    xq.dma_start(xcl_t[:, c], xcl[s][c])
                xlc_t = sb.tile([LV, NLV, C], F16, tag="xlc", bufs=3, name="xlc_t")
                for h in range(2):
                    nc.gpsimd.dma_start(
                        xlc_t[:, h * 7 : (h + 1) * 7, :],
                        xlc[s][:, h * 7 : (h + 1) * 7, :],
                    )
                state[s] = {"xcl": xcl_t, "xlc": xlc_t}

            def phase_a(s):
                st = state[s]
                xcl_t = st["xcl"]
                # qt = W1 @ xi + b1, output (C-chunk partition, HW free)
                qt = sb.tile([128, NCH, HWm], F16, tag="qt", bufs=2, name="qt")
                for pair in range(2):
                    qp = ps.tile([128, 2, HWm], F32, tag="pA", bufs=3, name="qp")
                    for i in range(2):
                        mk = 2 * pair + i
                        for cj in range(NCH):
                            nc.tensor.matmul(
                                qp[:, i, :],
                                w1_sb[:, cj, mk * 128 : (mk + 1) * 128],
                                xcl_t[:, 0, cj, 0:HWm],
                                start=(cj == 0),
                                stop=(cj == NCH - 1),
                            )
                    for i in range(2):
                        mk = 2 * pair + i
                        nc.scalar.activation(
                            qt[:, mk, :],
                            qp[:, i, :],
                            AF.Identity,
                            bias=b1_sb[:, mk : mk + 1],
                        )

                # scores qt^T x, chunked; per-chunk max + exp from PSUM
                p_t = sb.tile([MC, NMC, L], F16, tag="p", bufs=2, name="p_t")
                ncm = sb.tile([MC, NMC, NSC], F32, tag="ncm", bufs=2, name="ncm")
                rsums = sb.tile(
                    [MC, NMC, NSC], F32, tag="rsums", bufs=2, name="rsums"
                )
                for c in CHUNK_ORDER:
                    for mc in range(NMC):
                        sp = ps.tile([MC, SC], F32, tag="pA", bufs=3, name="sp")
                        for cj in range(NCH):
                            nc.tensor.matmul(
                                sp[:],
                                qt[:, cj, mc * MC : (mc + 1) * MC],
                                xcl_t[:, c, cj, :],
                                start=(cj == 0),
                                stop=(cj == NCH - 1),
                            )
                        nc.vector.reduce_max(
                            ncm[:, mc, c : c + 1], sp[:], axis=AX, negate=True
                        )
                        nc.scalar.activation(
                            p_t[:, mc, c * SC : (c + 1) * SC],
                            sp[:],
                            AF.Exp,
                            bias=ncm[:, mc, c : c + 1],
                            accum_out=rsums[:, mc, c : c + 1],
                        )

                st["p"] = p_t
                st["ncm"] = ncm
                st["rsums"] = rsums

            def emit_stats(s):
                # combine chunk stats: scl_c = exp(cmax_c - M) / rowsum.
                # Emitted AFTER phase_b(s-1) so these DVE/ACT ops (which wait
                # on exp(s)) never block the previous sample's scale/copies
                # in the in-order engine streams.
                st = state[s]
                ncm, rsums = st["ncm"], st["rsums"]
                cm2 = sb.tile([MC, NMC, NSC], F32, tag="cm2", bufs=2, name="cm2")
                gam = sb.tile([MC, NMC, NSC], F32, tag="gam", bufs=2, name="gam")
                prod = sb.tile([MC, NMC, NSC], F32, tag="prod", bufs=2, name="prod")
                stat = sb.tile([MC, NMC, 4], F32, tag="stat", bufs=2, name="stat")
                for mc in range(NMC):
                    nc.vector.tensor_scalar_mul(cm2[:, mc, :], ncm[:, mc, :], -1.0)
                    nc.vector.reduce_max(
                        stat[:, mc, 0:1], cm2[:, mc, :], axis=AX, negate=True
                    )
                    nc.scalar.activation(
                        gam[:, mc, :],
                        ncm[:, mc, :],
                        AF.Exp,
                        bias=stat[:, mc, 0:1],
                        scale=-1.0,
                    )
                    # tensor_tensor_reduce hangs real HW; use mul + reduce
                    nc.vector.tensor_mul(
                        prod[:, mc, :], gam[:, mc, :], rsums[:, mc, :]
                    )
                    nc.vector.reduce_sum(
                        stat[:, mc, 1:2], prod[:, mc, :], axis=AX
                    )
                    nc.vector.reciprocal(stat[:, mc, 2:3], stat[:, mc, 1:2])
                    nc.vector.tensor_scalar_mul(
                        gam[:, mc, :], gam[:, mc, :], stat[:, mc, 2:3]
                    )
                # normalize P in place per chunk: P *= gamma_c / rowsum
                # (per-partition scalar; PE transpose mode only passes data
                # through, so scaling must happen before the transposes).
                # Emitted HERE — after the previous sample's z/residual
                # copies and before the next sample's reduce_max ops — so
                # the in-order DVE stream never makes phase_b wait on it.
                p_t = st["p"]
                last = s == BC - 1
                for c in range(NSC):
                    for mc in range(NMC):
                        # on the last sample there is no next phase_a to hide
                        # this chain behind: split it across DVE and ACT
                        # (both idle at the tail) to halve its serial time
                        if last and (c + mc) % 2 == 1:
                            nc.scalar.mul(
                                p_t[:, mc, c * SC : (c + 1) * SC],
                                p_t[:, mc, c * SC : (c + 1) * SC],
                                gam[:, mc, c : c + 1],
                            )
                        else:
                            nc.vector.tensor_scalar_mul(
                                p_t[:, mc, c * SC : (c + 1) * SC],
                                p_t[:, mc, c * SC : (c + 1) * SC],
                                gam[:, mc, c : c + 1],
                            )

            def phase_b(s):
                st = state[s]
                p_t, xlc_t = st["p"], st["xlc"]
                # P^T via PE transpose (identity ifmap), buffered whole in
                # SBUF; Z = X P^T accumulated in two passes of 2 C-chunks so
                # each PSUM bank holds exactly one open accumulation group
                ptsb = sb.tile(
                    [LV, NLV, HWm], F16, tag="ptsb", bufs=2, name="ptsb"
                )
                z_sb = sb.tile([128, NCH, HWm], F16, tag="z", bufs=2, name="z_sb")
                ops = []
                for half in range(2):
                    zta = ps.tile([128, HWm], F32, tag="zt", bufs=2, name="zta")
                    ztb = ps.tile([128, HWm], F32, tag="zt", bufs=2, name="ztb")
                    for lp in range(NLV // 2):
                        if half == 0:
                            ptp = ps.tile(
                                [LV, 2, HWm], F16, tag="pB", bufs=3, name="ptp"
                            )
                            for j in range(2):
                                lc = 2 * lp + j
                                for mc in range(NMC):
                                    nc.tensor.transpose(
                                        ptp[:, j, mc * MC : (mc + 1) * MC],
                                        p_t[:, mc, lc * LV : (lc + 1) * LV],
                                        id_sb[:],
                                    )
                            nc.scalar.copy(
                                ptsb[:, 2 * lp : 2 * lp + 2, :], ptp[:]
                            )
                        elif lp == 2:
                            # open partial out-proj groups for cc 0,1 on the
                            # z chunks already drained from half 0: the PE
                            # work hides inside Z half 1, shortening the tail
                            for cc in range(2):
                                op = ps.tile(
                                    [128, HWm], F32, tag="pB", bufs=3, name="op"
                                )
                                for zj in range(2):
                                    nc.tensor.matmul(
                                        op[:],
                                        w2_sb[:, zj, cc * 128 : (cc + 1) * 128],
                                        z_sb[:, zj, :],
                                        start=(zj == 0),
                                        stop=False,
                                    )
                                ops.append(op)
                        for j in range(2):
                            lc = 2 * lp + j
                            for i, zt in ((0, zta), (1, ztb)):
                                cj = 2 * half + i
                                nc.tensor.matmul(
                                    zt[:],
                                    xlc_t[:, lc, cj * 128 : (cj + 1) * 128],
                                    ptsb[:, lc, :],
                                    start=(lc == 0),
                                    stop=(lc == NLV - 1),
                                )
                    for i, zt in ((0, zta), (1, ztb)):
                        cj = 2 * half + i
                        nc.vector.tensor_copy(z_sb[:, cj, :], zt[:])
                # close cc 0,1 with the second z half; cc 2,3 run whole
                for cc in range(2):
                    for zj in (2, 3):
                        nc.tensor.matmul(
                            ops[cc],
                            w2_sb[:, zj, cc * 128 : (cc + 1) * 128],
                            z_sb[:, zj, :],
                            start=False,
                            stop=(zj == NCH - 1),
                        )
                for cc in (2, 3):
                    op = ps.tile([128, HWm], F32, tag="pB", bufs=3, name="op")
                    for zj in range(NCH):
                        nc.tensor.matmul(
                            op[:],
                            w2_sb[:, zj, cc * 128 : (cc + 1) * 128],
                            z_sb[:, zj, :],
                            start=(zj == 0),
                            stop=(zj == NCH - 1),
                        )
                    ops.append(op)
                for pair in range(2):
                    osb = sb.tile([128, 2, HWm], F16, tag="osb", bufs=2, name="osb")
                    for i in range(2):
                        cc = 2 * pair + i
                        nc.vector.tensor_add(
                            osb[:, i, :], ops[cc][:], xib_sb[:, cc, s, :]
                        )
                    nc.sync.dma_start(
                        out[s].rearrange("(j p) m -> p j m", p=128)[
                            :, 2 * pair : 2 * pair + 2, :
                        ],
                        osb[:],
                    )
                del state[s]

            # prefetch one sample ahead: dispatches sit early in the engine
            # streams without flooding HBM with all samples at once
            emit_dma(0)
            emit_dma(1)
            for s in range(BC):
                if s + 2 < BC:
                    emit_dma(s + 2)
                phase_a(s)
                if s > 0:
                    phase_b(s - 1)
                emit_stats(s)
            phase_b(BC - 1)

    nc.compile()
    return nc


_NC = None


def _get_program():
    global _NC
    if _NC is None:
        _NC = build_program()
    return _NC


def make_in_maps(inputs):
    x_window = np.ascontiguousarray(np.asarray(inputs["x_window"], dtype=np.float32))
    Wq = np.asarray(inputs["Wq"], dtype=np.float32)
    bq_ = np.asarray(inputs["bq"], dtype=np.float32)
    Wk = np.asarray(inputs["Wk"], dtype=np.float32)
    Wv = np.asarray(inputs["Wv"], dtype=np.float32)
    bv_ = np.asarray(inputs["bv"], dtype=np.float32)
    Wo = np.asarray(inputs["Wo"], dtype=np.float32)
    bo_ = np.asarray(inputs["bo"], dtype=np.float32)

    xw = x_window.reshape(B, C, L)
    xib_full = (
        xw[:, :, CENT : CENT + HWm] + bo_[None, :, None] + (Wo @ bv_)[None, :, None]
    )

    def tile_w(wt):  # (in, out) -> [128, NCH, out] partition-major, fp16
        return np.ascontiguousarray(
            wt.reshape(NCH, 128, -1).transpose(1, 0, 2).astype(np.float16)
        )

    shared = {
        "w1T": tile_w(Wq.T @ Wk),
        "w2T": tile_w(Wv.T @ Wo.T),
        "b1": np.ascontiguousarray((Wk.T @ bq_).reshape(NCH, 128).T),
        "ident": np.eye(MC, dtype=np.float16),
    }
    in_maps = []
    for i in range(NCORES):
        m = dict(shared)
        xc = xw[i * BC : (i + 1) * BC]  # (BC, C, L)
        xp = np.concatenate(
            [
                xc[:, :, CENT : CENT + HWm],
                xc[:, :, :CENT],
                xc[:, :, CENT + HWm :],
            ],
            axis=2,
        ).astype(np.float16)  # l permuted center-first
        m["xcl"] = np.ascontiguousarray(
            xp.reshape(BC, NCH, 128, NSC, SC).transpose(0, 3, 2, 1, 4)
        )
        m["xlc"] = np.ascontiguousarray(
            xp.transpose(0, 2, 1).reshape(BC, NLV, LV, C).transpose(0, 2, 1, 3)
        )
        m["xib"] = np.ascontiguousarray(
            xib_full[i * BC : (i + 1) * BC]
            .reshape(BC, NCH, 128, HWm)
            .transpose(2, 1, 0, 3)
            .astype(np.float16)
        )
        in_maps.append(m)
    return in_maps


def run(inputs, trace=False, tmpdir=None):
    from concourse.bass_utils import run_bass_kernel_spmd

    nc = _get_program()
    in_maps = make_in_maps(inputs)
    res = run_bass_kernel_spmd(
        nc, in_maps, core_ids=list(range(NCORES)), trace=trace, tmpdir=tmpdir
    )
    outs = np.stack(
        [res.results[i]["out"].astype(np.float32) for i in range(NCORES)]
    )  # (8,4,C,HW)
    full = outs.reshape(B, C, HWm).reshape(B, C, 1, H, W)
    return full, res


def kernel(**inputs):
    full, _ = run(inputs)
    return full


# revision 42
# speedup vs baseline: 1.0722x; 1.0309x over previous
"""Bass/Tile TRN2 kernel for nn_LocalNodeAttentionHead.

Reference computation (per sample b):
    xi = x[:, :, t0]  (center frame)          (C, HW)
    xw = x reshaped                           (C, L)    L = T*H*W
    q  = Wq @ xi + bq                         (CI, HW)
    k  = Wk @ xw + bk                         (CI, L)
    v  = Wv @ xw + bv                         (L, CI)
    S  = q^T k  -> softmax over L             (HW, L)
    y  = softmax(S) @ v                       (CI, HW)
    out = Wo @ y + bo + xi                    (C, HW)

Algebraic restructure (host folds the weight products, fp32):
    S   = q^T Wk x = qt^T x    with qt = W1 xi + b1, W1 = Wk^T Wq, b1 = Wk^T bq
          (bk drops: constant along l under softmax)
    out = Wo (P V) + bo + xi = W2 (X P^T) + xib
          with W2 = Wo Wv, xib = xi + bo + Wo bv
          (rows of P sum to 1, so bv leaves the attention sum as a constant)
The k/v projections disappear entirely; per-sample device work is four
small GEMM groups: qt (16 mm), scores (28 mm), Z = X P^T (56 mm + 28 PE
transposes), out = W2 Z (16 mm).

Softmax runs chunked: each 441-wide score chunk takes its own max and
exponentiates straight out of PSUM; the cross-chunk correction
gamma_c = exp(cmax_c - M) and the 1/rowsum normalization combine into one
per-chunk scale applied to P in place on DVE before the PE transposes
(the PE transpose datapath only accepts permutation ifmaps).

Distribution: pure data-parallel, 4 samples per core on 8 cores.
All matmul operands fp16 (full PE rate at any free size), PSUM fp32,
softmax statistics fp32, residual fp16 (quantization ~2e-4 of scale).
x is shipped in both (C,L) and (L,C) layouts so the Z contraction needs
no on-chip transpose of x.
"""

import sys

sys.path.insert(0, "/opt/trn_rl_repo")

import numpy as np

import concourse.bass as bass
import concourse.tile as tile
from concourse import bacc, mybir

F32 = mybir.dt.float32
F16 = mybir.dt.float16
AF = mybir.ActivationFunctionType
AX = mybir.AxisListType.X
ALU = mybir.AluOpType

B, C, T, H, W = 32, 512, 9, 14, 14
CI = 512
HWm = H * W  # 196
L = T * HWm  # 1764
CENT = (T // 2) * HWm  # 784, center-frame offset in L
NCORES = 8
BC = B // NCORES  # 4 samples per core

NCH = C // 128  # 4 chunks of the channel dims
MC = 98  # query-row chunk (2 chunks of HW=196)
NMC = HWm // MC
SC = 441  # score l-chunk (PSUM fp32: 1764B < one bank)
NSC = L // SC  # 4
LV = 126  # l-chunk for P^T / Z (14 chunks; transpose out partitions)
NLV = L // LV
# The kv positions l are PERMUTED center-frame-first on the host (the
# attention math is order-invariant over l as long as x_cl, x_lc and P
# share the ordering): the qt projection reads one contiguous slice of
# DMA piece 0, and every x_cl piece is a fully contiguous HBM block.
CHUNK_ORDER = list(range(NSC))


def build_program():
    nc = bacc.Bacc("TRN2", target_bir_lowering=False, debug=False)

    xcl = nc.dram_tensor(
        "xcl", [BC, NSC, 128, NCH, SC], F16, kind="ExternalInput"
    ).ap()
    xlc = nc.dram_tensor("xlc", [BC, LV, NLV, C], F16, kind="ExternalInput").ap()
    w1T = nc.dram_tensor("w1T", [128, NCH, C], F16, kind="ExternalInput").ap()
    w2T = nc.dram_tensor("w2T", [128, NCH, C], F16, kind="ExternalInput").ap()
    b1 = nc.dram_tensor("b1", [128, NCH], F32, kind="ExternalInput").ap()
    xib = nc.dram_tensor("xib", [128, NCH, BC, HWm], F16, kind="ExternalInput").ap()
    ident = nc.dram_tensor("ident", [MC, MC], F16, kind="ExternalInput").ap()
    out = nc.dram_tensor("out", [BC, C, HWm], F16, kind="ExternalOutput").ap()

    with tile.TileContext(nc) as tc:
        with (
            tc.tile_pool(name="const", bufs=1) as const,
            tc.tile_pool(name="sb", bufs=1) as sb,
            tc.tile_pool(name="ps", bufs=1, space="PSUM") as ps,
        ):
            # ---- constants: qt-path weights lead the SP queue; the
            # phase_b-only constants head the pool queue ahead of x_lc
            w1_sb = const.tile([128, NCH, C], F16)
            nc.sync.dma_start(w1_sb[:], w1T[:])
            b1_sb = const.tile([128, NCH], F32)
            nc.sync.dma_start(b1_sb[:], b1[:])
            id_sb = const.tile([MC, MC], F16)
            nc.gpsimd.dma_start(id_sb[:], ident[:])
            w2_sb = const.tile([128, NCH, C], F16)
            nc.gpsimd.dma_start(w2_sb[:], w2T[:])
            xib_sb = const.tile([128, NCH, BC, HWm], F16)
            nc.gpsimd.dma_start(xib_sb[:], xib[:])

            state = {}

            def emit_dma(s):
                # interleave each sample's pieces across both the SP and
                # ACT DMA queues so every sample's x arrives in roughly half
                # the single-queue serial time (helps the startup samples)
                xcl_t = sb.tile(
                    [128, NSC, NCH, SC], F16, tag="xcl", bufs=3, name="xcl_t"
                )
                for ci, c in enumerate(CHUNK_ORDER):
                    xq = nc.sync if ci % 2 == 0 else nc.scalar
                    xq.dma_start(xcl_t[:, c], xcl[s][c])
                xlc_t = sb.tile([LV, NLV, C], F16, tag="xlc", bufs=3, name="xlc_t")
                # quarters rather than halves: the Z loop consumes lc chunks
                # in order, so finer pieces shrink the stall when a sample's
                # x_lc tail is still streaming as phase_b reaches it
                for a, b in ((0, 4), (4, 8), (8, 11), (11, 14)):
                    nc.gpsimd.dma_start(
                        xlc_t[:, a:b, :], xlc[s][:, a:b, :]
                    )
                state[s] = {"xcl": xcl_t, "xlc": xlc_t}

            def phase_a(s):
                st = state[s]
                xcl_t = st["xcl"]
                # qt = W1 @ xi + b1, output (C-chunk partition, HW free)
                qt = sb.tile([128, NCH, HWm], F16, tag="qt", bufs=2, name="qt")
                for pair in range(2):
                    qp = ps.tile([128, 2, HWm], F32, tag="pA", bufs=3, name="qp")
                    for i in range(2):
                        mk = 2 * pair + i
                        for cj in range(NCH):
                            nc.tensor.matmul(
                                qp[:, i, :],
                                w1_sb[:, cj, mk * 128 : (mk + 1) * 128],
                                xcl_t[:, 0, cj, 0:HWm],
                                start=(cj == 0),
                                stop=(cj == NCH - 1),
                            )
                    for i in range(2):
                        mk = 2 * pair + i
                        nc.scalar.activation(
                            qt[:, mk, :],
                            qp[:, i, :],
                            AF.Identity,
                            bias=b1_sb[:, mk : mk + 1],
                        )

                # scores qt^T x, chunked; per-chunk max + exp from PSUM
                p_t = sb.tile([MC, NMC, L], F16, tag="p", bufs=2, name="p_t")
                ncm = sb.tile([MC, NMC, NSC], F32, tag="ncm", bufs=2, name="ncm")
                rsums = sb.tile(
                    [MC, NMC, NSC], F32, tag="rsums", bufs=2, name="rsums"
                )
                for c in CHUNK_ORDER:
                    for mc in range(NMC):
                        sp = ps.tile([MC, SC], F32, tag="pA", bufs=3, name="sp")
                        for cj in range(NCH):
                            nc.tensor.matmul(
                                sp[:],
                                qt[:, cj, mc * MC : (mc + 1) * MC],
                                xcl_t[:, c, cj, :],
                                start=(cj == 0),
                                stop=(cj == NCH - 1),
                            )
                        nc.vector.reduce_max(
                            ncm[:, mc, c : c + 1], sp[:], axis=AX, negate=True
                        )
                        nc.scalar.activation(
                            p_t[:, mc, c * SC : (c + 1) * SC],
                            sp[:],
                            AF.Exp,
                            bias=ncm[:, mc, c : c + 1],
                            accum_out=rsums[:, mc, c : c + 1],
                        )

                st["p"] = p_t
                st["ncm"] = ncm
                st["rsums"] = rsums

            def emit_stats(s):
                # combine chunk stats: scl_c = exp(cmax_c - M) / rowsum.
                # Emitted AFTER phase_b(s-1) so these DVE/ACT ops (which wait
                # on exp(s)) never block the previous sample's scale/copies
                # in the in-order engine streams.
                st = state[s]
                ncm, rsums = st["ncm"], st["rsums"]
                cm2 = sb.tile([MC, NMC, NSC], F32, tag="cm2", bufs=2, name="cm2")
                gam = sb.tile([MC, NMC, NSC], F32, tag="gam", bufs=2, name="gam")
                prod = sb.tile([MC, NMC, NSC], F32, tag="prod", bufs=2, name="prod")
                stat = sb.tile([MC, NMC, 4], F32, tag="stat", bufs=2, name="stat")
                for mc in range(NMC):
                    nc.vector.tensor_scalar_mul(cm2[:, mc, :], ncm[:, mc, :], -1.0)
                    nc.vector.reduce_max(
                        stat[:, mc, 0:1], cm2[:, mc, :], axis=AX, negate=True
                    )
                    nc.scalar.activation(
                        gam[:, mc, :],
                        ncm[:, mc, :],
                        AF.Exp,
                        bias=stat[:, mc, 0:1],
                        scale=-1.0,
                    )
                    # tensor_tensor_reduce hangs real HW; use mul + reduce
                    nc.vector.tensor_mul(
                        prod[:, mc, :], gam[:, mc, :], rsums[:, mc, :]
                    )
                    nc.vector.reduce_sum(
                        stat[:, mc, 1:2], prod[:, mc, :], axis=AX
                    )
                    nc.vector.reciprocal(stat[:, mc, 2:3], stat[:, mc, 1:2])
                    nc.vector.tensor_scalar_mul(
                        gam[:, mc, :], gam[:, mc, :], stat[:, mc, 2:3]
                    )
                # normalize P in place per chunk: P *= gamma_c / rowsum
                # (per-partition scalar; PE transpose mode only passes data
                # through, so scaling must happen before the transposes).
                # Emitted HERE — after the previous sample's z/residual
                # copies and before the next sample's reduce_max ops — so
                # the in-order DVE stream never makes phase_b wait on it.
                p_t = st["p"]
                last = s == BC - 1
                for c in range(NSC):
                    for mc in range(NMC):
                        # on the last sample there is no next phase_a to hide
                        # this chain behind: split it across DVE and ACT
                        # (both idle at the tail) to halve its serial time
                        if last and (c + mc) % 2 == 1:
                            nc.scalar.mul(
                                p_t[:, mc, c * SC : (c + 1) * SC],
                                p_t[:, mc, c * SC : (c + 1) * SC],
                                gam[:, mc, c : c + 1],
                            )
                        else:
                            nc.vector.tensor_scalar_mul(
                                p_t[:, mc, c * SC : (c + 1) * SC],
                                p_t[:, mc, c * SC : (c + 1) * SC],
                                gam[:, mc, c : c + 1],
                            )

            def phase_b(s):
                st = state[s]
                p_t, xlc_t = st["p"], st["xlc"]
                # P^T via PE transpose (identity ifmap), buffered whole in
                # SBUF; Z = X P^T accumulated in two passes of 2 C-chunks so
                # each PSUM bank holds exactly one open accumulation group
                ptsb = sb.tile(
                    [LV, NLV, HWm], F16, tag="ptsb", bufs=2, name="ptsb"
                )
                z_sb = sb.tile([128, NCH, HWm], F16, tag="z", bufs=2, name="z_sb")
                ops = []
                for half in range(2):
                    zta = ps.tile([128, HWm], F32, tag="zt", bufs=2, name="zta")
                    ztb = ps.tile([128, HWm], F32, tag="zt", bufs=2, name="ztb")
                    for lp in range(NLV // 2):
                        if half == 0:
                            ptp = ps.tile(
                                [LV, 2, HWm], F16, tag="pB", bufs=3, name="ptp"
                            )
                            for j in range(2):
                                lc = 2 * lp + j
                                for mc in range(NMC):
                                    nc.tensor.transpose(
                                        ptp[:, j, mc * MC : (mc + 1) * MC],
                                        p_t[:, mc, lc * LV : (lc + 1) * LV],
                                        id_sb[:],
                                    )
                            nc.scalar.copy(
                                ptsb[:, 2 * lp : 2 * lp + 2, :], ptp[:]
                            )
                        elif lp == 2:
                            # open partial out-proj groups for cc 0,1 on the
                            # z chunks already drained from half 0: the PE
                            # work hides inside Z half 1, shortening the tail
                            for cc in range(2):
                                op = ps.tile(
                                    [128, HWm], F32, tag="pB", bufs=3, name="op"
                                )
                                for zj in range(2):
                                    nc.tensor.matmul(
                                        op[:],
                                        w2_sb[:, zj, cc * 128 : (cc + 1) * 128],
                                        z_sb[:, zj, :],
                                        start=(zj == 0),
                                        stop=False,
                                    )
                                ops.append(op)
                        for j in range(2):
                            lc = 2 * lp + j
                            for i, zt in ((0, zta), (1, ztb)):
                                cj = 2 * half + i
                                nc.tensor.matmul(
                                    zt[:],
                                    xlc_t[:, lc, cj * 128 : (cj + 1) * 128],
                                    ptsb[:, lc, :],
                                    start=(lc == 0),
                                    stop=(lc == NLV - 1),
                                )
                    for i, zt in ((0, zta), (1, ztb)):
                        cj = 2 * half + i
                        nc.vector.tensor_copy(z_sb[:, cj, :], zt[:])
                # close cc 0,1 with the second z half; cc 2,3 run whole
                for cc in range(2):
                    for zj in (2, 3):
                        nc.tensor.matmul(
                            ops[cc],
                            w2_sb[:, zj, cc * 128 : (cc + 1) * 128],
                            z_sb[:, zj, :],
                            start=False,
                            stop=(zj == NCH - 1),
                        )
                for cc in (2, 3):
                    op = ps.tile([128, HWm], F32, tag="pB", bufs=3, name="op")
                    for zj in range(NCH):
                        nc.tensor.matmul(
                            op[:],
                            w2_sb[:, zj, cc * 128 : (cc + 1) * 128],
                            z_sb[:, zj, :],
                            start=(zj == 0),
                            stop=(zj == NCH - 1),
                        )
                    ops.append(op)
                for pair in range(2):
                    osb = sb.tile([128, 2, HWm], F16, tag="osb", bufs=2, name="osb")
                    for i in range(2):
                        cc = 2 * pair + i
                        nc.vector.tensor_add(
                            osb[:, i, :], ops[cc][:], xib_sb[:, cc, s, :]
                        )
                    nc.sync.dma_start(
                        out[s].rearrange("(j p) m -> p j m", p=128)[
                            :, 2 * pair : 2 * pair + 2, :
                        ],
                        osb[:],
                    )
                del state[s]

            # prefetch one sample ahead: dispatches sit early in the engine
            # streams without flooding HBM with all samples at once
            emit_dma(0)
            emit_dma(1)
            for s in range(BC):
                if s + 2 < BC:
                    emit_dma(s + 2)
                phase_a(s)
                if s > 0:
                    phase_b(s - 1)
                emit_stats(s)
            phase_b(BC - 1)

    nc.compile()
    return nc


_NC = None


def _get_program():
    global _NC
    if _NC is None:
        _NC = build_program()
    return _NC


def make_in_maps(inputs):
    x_window = np.ascontiguousarray(np.asarray(inputs["x_window"], dtype=np.float32))
    Wq = np.asarray(inputs["Wq"], dtype=np.float32)
    bq_ = np.asarray(inputs["bq"], dtype=np.float32)
    Wk = np.asarray(inputs["Wk"], dtype=np.float32)
    Wv = np.asarray(inputs["Wv"], dtype=np.float32)
    bv_ = np.asarray(inputs["bv"], dtype=np.float32)
    Wo = np.asarray(inputs["Wo"], dtype=np.float32)
    bo_ = np.asarray(inputs["bo"], dtype=np.float32)

    xw = x_window.reshape(B, C, L)
    xib_full = (
        xw[:, :, CENT : CENT + HWm] + bo_[None, :, None] + (Wo @ bv_)[None, :, None]
    )

    def tile_w(wt):  # (in, out) -> [128, NCH, out] partition-major, fp16
        return np.ascontiguousarray(
            wt.reshape(NCH, 128, -1).transpose(1, 0, 2).astype(np.float16)
        )

    shared = {
        "w1T": tile_w(Wq.T @ Wk),
        "w2T": tile_w(Wv.T @ Wo.T),
        "b1": np.ascontiguousarray((Wk.T @ bq_).reshape(NCH, 128).T),
        "ident": np.eye(MC, dtype=np.float16),
    }
    in_maps = []
    for i in range(NCORES):
        m = dict(shared)
        xc = xw[i * BC : (i + 1) * BC]  # (BC, C, L)
        xp = np.concatenate(
            [
                xc[:, :, CENT : CENT + HWm],
                xc[:, :, :CENT],
                xc[:, :, CENT + HWm :],
            ],
            axis=2,
        ).astype(np.float16)  # l permuted center-first
        m["xcl"] = np.ascontiguousarray(
            xp.reshape(BC, NCH, 128, NSC, SC).transpose(0, 3, 2, 1, 4)
        )
        m["xlc"] = np.ascontiguousarray(
            xp.transpose(0, 2, 1).reshape(BC, NLV, LV, C).transpose(0, 2, 1, 3)
        )
        m["xib"] = np.ascontiguousarray(
            xib_full[i * BC : (i + 1) * BC]
            .reshape(BC, NCH, 128, HWm)
            .transpose(2, 1, 0, 3)
            .astype(np.float16)
        )
        in_maps.append(m)
    return in_maps


def run(inputs, trace=False, tmpdir=None):
    from concourse.bass_utils import run_bass_kernel_spmd

    nc = _get_program()
    in_maps = make_in_maps(inputs)
    res = run_bass_kernel_spmd(
        nc, in_maps, core_ids=list(range(NCORES)), trace=trace, tmpdir=tmpdir
    )
    outs = np.stack(
        [res.results[i]["out"].astype(np.float32) for i in range(NCORES)]
    )  # (8,4,C,HW)
    full = outs.reshape(B, C, HWm).reshape(B, C, 1, H, W)
    return full, res


def kernel(**inputs):
    full, _ = run(inputs)
    return full


# revision 44
# speedup vs baseline: 1.1099x; 1.0352x over previous
"""Bass/Tile TRN2 kernel for nn_LocalNodeAttentionHead.

Reference computation (per sample b):
    xi = x[:, :, t0]  (center frame)          (C, HW)
    xw = x reshaped                           (C, L)    L = T*H*W
    q  = Wq @ xi + bq                         (CI, HW)
    k  = Wk @ xw + bk                         (CI, L)
    v  = Wv @ xw + bv                         (L, CI)
    S  = q^T k  -> softmax over L             (HW, L)
    y  = softmax(S) @ v                       (CI, HW)
    out = Wo @ y + bo + xi                    (C, HW)

Algebraic restructure (host folds the weight products, fp32):
    S   = q^T Wk x = qt^T x    with qt = W1 xi + b1, W1 = Wk^T Wq, b1 = Wk^T bq
          (bk drops: constant along l under softmax)
    out = Wo (P V) + bo + xi = W2 (X P^T) + xib
          with W2 = Wo Wv, xib = xi + bo + Wo bv
          (rows of P sum to 1, so bv leaves the attention sum as a constant)
The k/v projections disappear entirely; per-sample device work is four
small GEMM groups: qt (16 mm), scores (28 mm), Z = X P^T (56 mm + 28 PE
transposes), out = W2 Z (16 mm).

Softmax runs chunked: each 441-wide score chunk takes its own max and
exponentiates straight out of PSUM; the cross-chunk correction
gamma_c = exp(cmax_c - M) and the 1/rowsum normalization combine into one
per-chunk scale applied to P in place on DVE before the PE transposes
(the PE transpose datapath only accepts permutation ifmaps).

Distribution: pure data-parallel, 4 samples per core on 8 cores.
All matmul operands fp16 (full PE rate at any free size), PSUM fp32,
softmax statistics fp32, residual fp16 (quantization ~2e-4 of scale).
x is shipped in both (C,L) and (L,C) layouts so the Z contraction needs
no on-chip transpose of x.
"""

import sys

sys.path.insert(0, "/opt/trn_rl_repo")

import numpy as np

import concourse.bass as bass
import concourse.tile as tile
from concourse import bacc, mybir

F32 = mybir.dt.float32
F16 = mybir.dt.float16
AF = mybir.ActivationFunctionType
AX = mybir.AxisListType.X
ALU = mybir.AluOpType

B, C, T, H, W = 32, 512, 9, 14, 14
CI = 512
HWm = H * W  # 196
L = T * HWm  # 1764
CENT = (T // 2) * HWm  # 784, center-frame offset in L
NCORES = 8
BC = B // NCORES  # 4 samples per core

NCH = C // 128  # 4 chunks of the channel dims
MC = 98  # query-row chunk (2 chunks of HW=196)
NMC = HWm // MC
SC = 441  # score l-chunk (PSUM fp32: 1764B < one bank)
NSC = L // SC  # 4
LV = 126  # l-chunk for P^T / Z (14 chunks; transpose out partitions)
NLV = L // LV
# The kv positions l are PERMUTED center-frame-first on the host (the
# attention math is order-invariant over l as long as x_cl, x_lc and P
# share the ordering): the qt projection reads one contiguous slice of
# DMA piece 0, and every x_cl piece is a fully contiguous HBM block.
CHUNK_ORDER = list(range(NSC))


def build_program():
    nc = bacc.Bacc("TRN2", target_bir_lowering=False, debug=False)

    xcl = nc.dram_tensor(
        "xcl", [BC, NSC, 128, NCH, SC], F16, kind="ExternalInput"
    ).ap()
    xlc = nc.dram_tensor("xlc", [BC, LV, NLV, C], F16, kind="ExternalInput").ap()
    w1T = nc.dram_tensor("w1T", [128, NCH, C], F16, kind="ExternalInput").ap()
    w2T = nc.dram_tensor("w2T", [128, NCH, C], F16, kind="ExternalInput").ap()
    b1 = nc.dram_tensor("b1", [128, NCH], F32, kind="ExternalInput").ap()
    xib = nc.dram_tensor("xib", [128, NCH, BC, HWm], F16, kind="ExternalInput").ap()
    ident = nc.dram_tensor("ident", [MC, MC], F16, kind="ExternalInput").ap()
    out = nc.dram_tensor("out", [BC, C, HWm], F16, kind="ExternalOutput").ap()

    with tile.TileContext(nc) as tc:
        with (
            tc.tile_pool(name="const", bufs=1) as const,
            tc.tile_pool(name="sb", bufs=1) as sb,
            tc.tile_pool(name="ps", bufs=1, space="PSUM") as ps,
        ):
            # ---- constants: qt-path weights lead the SP queue; the
            # phase_b-only constants head the pool queue ahead of x_lc
            w1_sb = const.tile([128, NCH, C], F16)
            nc.sync.dma_start(w1_sb[:], w1T[:])
            b1_sb = const.tile([128, NCH], F32)
            nc.sync.dma_start(b1_sb[:], b1[:])
            id_sb = const.tile([MC, MC], F16)
            nc.gpsimd.dma_start(id_sb[:], ident[:])
            w2_sb = const.tile([128, NCH, C], F16)
            nc.gpsimd.dma_start(w2_sb[:], w2T[:])
            xib_sb = const.tile([128, NCH, BC, HWm], F16)
            nc.gpsimd.dma_start(xib_sb[:], xib[:])

            state = {}

            def emit_dma(s):
                # interleave each sample's pieces across both the SP and
                # ACT DMA queues so every sample's x arrives in roughly half
                # the single-queue serial time (helps the startup samples)
                xcl_t = sb.tile(
                    [128, NSC, NCH, SC], F16, tag="xcl", bufs=3, name="xcl_t"
                )
                for ci, c in enumerate(CHUNK_ORDER):
                    xq = nc.sync if ci % 2 == 0 else nc.scalar
                    xq.dma_start(xcl_t[:, c], xcl[s][c])
                xlc_t = sb.tile([LV, NLV, C], F16, tag="xlc", bufs=3, name="xlc_t")
                # quarters rather than halves: the Z loop consumes lc chunks
                # in order, so finer pieces shrink the stall when a sample's
                # x_lc tail is still streaming as phase_b reaches it
                for a, b in ((0, 4), (4, 8), (8, 11), (11, 14)):
                    nc.gpsimd.dma_start(
                        xlc_t[:, a:b, :], xlc[s][:, a:b, :]
                    )
                state[s] = {"xcl": xcl_t, "xlc": xlc_t}

            def phase_a(s):
                st = state[s]
                xcl_t = st["xcl"]
                # qt = W1 @ xi + b1, output (C-chunk partition, HW free)
                qt = sb.tile([128, NCH, HWm], F16, tag="qt", bufs=2, name="qt")
                for pair in range(2):
                    qp = ps.tile([128, 2, HWm], F32, tag="pA", bufs=3, name="qp")
                    for i in range(2):
                        mk = 2 * pair + i
                        for cj in range(NCH):
                            nc.tensor.matmul(
                                qp[:, i, :],
                                w1_sb[:, cj, mk * 128 : (mk + 1) * 128],
                                xcl_t[:, 0, cj, 0:HWm],
                                start=(cj == 0),
                                stop=(cj == NCH - 1),
                            )
                    for i in range(2):
                        mk = 2 * pair + i
                        nc.scalar.activation(
                            qt[:, mk, :],
                            qp[:, i, :],
                            AF.Identity,
                            bias=b1_sb[:, mk : mk + 1],
                        )

                # scores qt^T x, chunked; per-chunk max + exp from PSUM
                p_t = sb.tile([MC, NMC, L], F16, tag="p", bufs=2, name="p_t")
                ncm = sb.tile([MC, NMC, NSC], F32, tag="ncm", bufs=2, name="ncm")
                rsums = sb.tile(
                    [MC, NMC, NSC], F32, tag="rsums", bufs=2, name="rsums"
                )
                for c in CHUNK_ORDER:
                    for mc in range(NMC):
                        sp = ps.tile([MC, SC], F32, tag="pA", bufs=3, name="sp")
                        for cj in range(NCH):
                            nc.tensor.matmul(
                                sp[:],
                                qt[:, cj, mc * MC : (mc + 1) * MC],
                                xcl_t[:, c, cj, :],
                                start=(cj == 0),
                                stop=(cj == NCH - 1),
                            )
                        nc.vector.reduce_max(
                            ncm[:, mc, c : c + 1], sp[:], axis=AX, negate=True
                        )
                        nc.scalar.activation(
                            p_t[:, mc, c * SC : (c + 1) * SC],
                            sp[:],
                            AF.Exp,
                            bias=ncm[:, mc, c : c + 1],
                            accum_out=rsums[:, mc, c : c + 1],
                        )

                st["p"] = p_t
                st["ncm"] = ncm
                st["rsums"] = rsums

            def emit_stats(s):
                # combine chunk stats: scl_c = exp(cmax_c - M) / rowsum.
                # Emitted AFTER phase_b(s-1) so these DVE/ACT ops (which wait
                # on exp(s)) never block the previous sample's scale/copies
                # in the in-order engine streams.
                st = state[s]
                ncm, rsums = st["ncm"], st["rsums"]
                cm2 = sb.tile([MC, NMC, NSC], F32, tag="cm2", bufs=2, name="cm2")
                gam = sb.tile([MC, NMC, NSC], F32, tag="gam", bufs=2, name="gam")
                prod = sb.tile([MC, NMC, NSC], F32, tag="prod", bufs=2, name="prod")
                stat = sb.tile([MC, NMC, 4], F32, tag="stat", bufs=2, name="stat")
                for mc in range(NMC):
                    nc.vector.tensor_scalar_mul(cm2[:, mc, :], ncm[:, mc, :], -1.0)
                    nc.vector.reduce_max(
                        stat[:, mc, 0:1], cm2[:, mc, :], axis=AX, negate=True
                    )
                    nc.scalar.activation(
                        gam[:, mc, :],
                        ncm[:, mc, :],
                        AF.Exp,
                        bias=stat[:, mc, 0:1],
                        scale=-1.0,
                    )
                    # tensor_tensor_reduce hangs real HW; use mul + reduce
                    nc.vector.tensor_mul(
                        prod[:, mc, :], gam[:, mc, :], rsums[:, mc, :]
                    )
                    nc.vector.reduce_sum(
                        stat[:, mc, 1:2], prod[:, mc, :], axis=AX
                    )
                    nc.vector.reciprocal(stat[:, mc, 2:3], stat[:, mc, 1:2])
                    nc.vector.tensor_scalar_mul(
                        gam[:, mc, :], gam[:, mc, :], stat[:, mc, 2:3]
                    )
                # normalize P in place per chunk: P *= gamma_c / rowsum
                # (per-partition scalar; PE transpose mode only passes data
                # through, so scaling must happen before the transposes).
                # Emitted HERE — after the previous sample's z/residual
                # copies and before the next sample's reduce_max ops — so
                # the in-order DVE stream never makes phase_b wait on it.
                p_t = st["p"]
                last = s == BC - 1
                for c in range(NSC):
                    for mc in range(NMC):
                        # on the last sample there is no next phase_a to hide
                        # this chain behind: split it across DVE and ACT
                        # (both idle at the tail) to halve its serial time
                        if last and (c + mc) % 2 == 1:
                            nc.scalar.mul(
                                p_t[:, mc, c * SC : (c + 1) * SC],
                                p_t[:, mc, c * SC : (c + 1) * SC],
                                gam[:, mc, c : c + 1],
                            )
                        else:
                            nc.vector.tensor_scalar_mul(
                                p_t[:, mc, c * SC : (c + 1) * SC],
                                p_t[:, mc, c * SC : (c + 1) * SC],
                                gam[:, mc, c : c + 1],
                            )

            def phase_b(s):
                st = state[s]
                p_t, xlc_t = st["p"], st["xlc"]
                # P^T via PE transpose (identity ifmap), buffered whole in
                # SBUF; Z = X P^T accumulated in two passes of 2 C-chunks so
                # each PSUM bank holds exactly one open accumulation group
                ptsb = sb.tile(
                    [LV, NLV, HWm], F16, tag="ptsb", bufs=2, name="ptsb"
                )
                z_sb = sb.tile([128, NCH, HWm], F16, tag="z", bufs=2, name="z_sb")
                ops = []
                for half in range(2):
                    zta = ps.tile([128, HWm], F32, tag="zt", bufs=2, name="zta")
                    ztb = ps.tile([128, HWm], F32, tag="zt", bufs=2, name="ztb")
                    for lp in range(NLV // 2):
                        if half == 0:
                            ptp = ps.tile(
                                [LV, 2, HWm], F16, tag="pB", bufs=3, name="ptp"
                            )
                            for j in range(2):
                                lc = 2 * lp + j
                                for mc in range(NMC):
                                    nc.tensor.transpose(
                                        ptp[:, j, mc * MC : (mc + 1) * MC],
                                        p_t[:, mc, lc * LV : (lc + 1) * LV],
                                        id_sb[:],
                                    )
                            nc.scalar.copy(
                                ptsb[:, 2 * lp : 2 * lp + 2, :], ptp[:]
                            )
                        elif lp == 2:
                            # open partial out-proj groups for cc 0,1 on the
                            # z chunks already drained from half 0: the PE
                            # work hides inside Z half 1, shortening the tail
                            for cc in range(2):
                                op = ps.tile(
                                    [128, HWm], F32, tag="pB", bufs=3, name="op"
                                )
                                for zj in range(2):
                                    nc.tensor.matmul(
                                        op[:],
                                        w2_sb[:, zj, cc * 128 : (cc + 1) * 128],
                                        z_sb[:, zj, :],
                                        start=(zj == 0),
                                        stop=False,
                                    )
                                ops.append(op)
                        for j in range(2):
                            lc = 2 * lp + j
                            for i, zt in ((0, zta), (1, ztb)):
                                cj = 2 * half + i
                                nc.tensor.matmul(
                                    zt[:],
                                    xlc_t[:, lc, cj * 128 : (cj + 1) * 128],
                                    ptsb[:, lc, :],
                                    start=(lc == 0),
                                    stop=(lc == NLV - 1),
                                )
                    for i, zt in ((0, zta), (1, ztb)):
                        cj = 2 * half + i
                        nc.vector.tensor_copy(z_sb[:, cj, :], zt[:])
                # close cc 0,1 with the second z half; cc 2,3 run whole
                for cc in range(2):
                    for zj in (2, 3):
                        nc.tensor.matmul(
                            ops[cc],
                            w2_sb[:, zj, cc * 128 : (cc + 1) * 128],
                            z_sb[:, zj, :],
                            start=False,
                            stop=(zj == NCH - 1),
                        )
                for cc in (2, 3):
                    op = ps.tile([128, HWm], F32, tag="pB", bufs=3, name="op")
                    for zj in range(NCH):
                        nc.tensor.matmul(
                            op[:],
                            w2_sb[:, zj, cc * 128 : (cc + 1) * 128],
                            z_sb[:, zj, :],
                            start=(zj == 0),
                            stop=(zj == NCH - 1),
                        )
                    ops.append(op)
                for pair in range(2):
                    osb = sb.tile([128, 2, HWm], F16, tag="osb", bufs=2, name="osb")
                    for i in range(2):
                        cc = 2 * pair + i
                        nc.vector.tensor_add(
                            osb[:, i, :], ops[cc][:], xib_sb[:, cc, s, :]
                        )
                    nc.sync.dma_start(
                        out[s].rearrange("(j p) m -> p j m", p=128)[
                            :, 2 * pair : 2 * pair + 2, :
                        ],
                        osb[:],
                    )
                del state[s]

            # prefetch one sample ahead: dispatches sit early in the engine
            # streams without flooding HBM with all samples at once
            emit_dma(0)
            emit_dma(1)
            for s in range(BC):
                if s + 2 < BC:
                    emit_dma(s + 2)
                phase_a(s)
                if s > 0:
                    phase_b(s - 1)
                emit_stats(s)
            phase_b(BC - 1)

    nc.compile()
    return nc


_NC = None


def _get_program():
    global _NC
    if _NC is None:
        _NC = build_program()
    return _NC


def make_in_maps(inputs):
    x_window = np.ascontiguousarray(np.asarray(inputs["x_window"], dtype=np.float32))
    Wq = np.asarray(inputs["Wq"], dtype=np.float32)
    bq_ = np.asarray(inputs["bq"], dtype=np.float32)
    Wk = np.asarray(inputs["Wk"], dtype=np.float32)
    Wv = np.asarray(inputs["Wv"], dtype=np.float32)
    bv_ = np.asarray(inputs["bv"], dtype=np.float32)
    Wo = np.asarray(inputs["Wo"], dtype=np.float32)
    bo_ = np.asarray(inputs["bo"], dtype=np.float32)

    xw = x_window.reshape(B, C, L)
    xib_full = (
        xw[:, :, CENT : CENT + HWm] + bo_[None, :, None] + (Wo @ bv_)[None, :, None]
    )

    def tile_w(wt):  # (in, out) -> [128, NCH, out] partition-major, fp16
        return np.ascontiguousarray(
            wt.reshape(NCH, 128, -1).transpose(1, 0, 2).astype(np.float16)
        )

    shared = {
        "w1T": tile_w(Wq.T @ Wk),
        "w2T": tile_w(Wv.T @ Wo.T),
        "b1": np.ascontiguousarray((Wk.T @ bq_).reshape(NCH, 128).T),
        "ident": np.eye(MC, dtype=np.float16),
    }
    in_maps = []
    for i in range(NCORES):
        m = dict(shared)
        xc = xw[i * BC : (i + 1) * BC]  # (BC, C, L)
        xp = np.concatenate(
            [
                xc[:, :, CENT : CENT + HWm],
                xc[:, :, :CENT],
                xc[:, :, CENT + HWm :],
            ],
            axis=2,
        ).astype(np.float16)  # l permuted center-first
        m["xcl"] = np.ascontiguousarray(
            xp.reshape(BC, NCH, 128, NSC, SC).transpose(0, 3, 2, 1, 4)
        )
        m["xlc"] = np.ascontiguousarray(
            xp.transpose(0, 2, 1).reshape(BC, NLV, LV, C).transpose(0, 2, 1, 3)
        )
        m["xib"] = np.ascontiguousarray(
            xib_full[i * BC : (i + 1) * BC]
            .reshape(BC, NCH, 128, HWm)
            .transpose(2, 1, 0, 3)
            .astype(np.float16)
        )
        in_maps.append(m)
    return in_maps


def run(inputs, trace=False, tmpdir=None):
    from concourse.bass_utils import run_bass_kernel_spmd

    nc = _get_program()
    in_maps = make_in_maps(inputs)
    res = run_bass_kernel_spmd(
        nc, in_maps, core_ids=list(range(NCORES)), trace=trace, tmpdir=tmpdir
    )
    outs = np.stack(
        [res.results[i]["out"].astype(np.float32) for i in range(NCORES)]
    )  # (8,4,C,HW)
    full = outs.reshape(B, C, HWm).reshape(B, C, 1, H, W)
    return full, res


def kernel(**inputs):
    full, _ = run(inputs)
    return full
